# revision 56
# baseline (speedup 1.0000x reference)
"""Log-normal NLL loss kernel for Trainium2 (8 NeuronCores, data-parallel).

Reference math (per sample):
    preds = (mu1, log_sigma1, mu2, log_sigma2); y = truth
    s   = exp(2*log_sigma1) + exp(2*log_sigma2)          # sigma1^2 + sigma2^2
    mu  = mu1 + mu2
    out = log(y) + 0.5*log(2*pi*s) + (log(y) - mu)^2 / (2*s)

Batch is split evenly across the 8 cores (no communication).  The per-core
program (build_nc_raw) is raw bass with hand-placed semaphores: a 4-stage
software pipeline (DMA-load / ACT / DVE / store-on-GpSimd) with 3-deep
load prefetch.  The division by s runs on ACT via exp/-ln
(v = Square(z*sqrt(pi)) * exp(-ln(2*pi*s)) = z^2/(2*s)): 6 ACT passes +
6 DVE passes per element, balanced against the HBM roofline
(~24 MB/core @ ~358 GB/s ~= 67 us; measured ~92-108 us end to end).

build_nc is an earlier TileContext implementation kept as a reference /
fallback (~108-115 us).
"""

import os
import sys

import numpy as np

for _p in ("/opt/trn_rl_repo", os.path.expanduser("~/.axon_site/_ro/trn_rl_repo")):
    if os.path.isdir(_p) and _p not in sys.path:
        sys.path.insert(0, _p)

import concourse.bacc as bacc
import concourse.bass as bass
import concourse.mybir as mybir
import concourse.tile as tile
from concourse.bass_utils import run_bass_kernel_spmd

B = 8388608
N_CORES = 8
BC = B // N_CORES  # 1048576 samples per core
P = 128            # SBUF partitions

LOG_2PI = float(np.log(2.0 * np.pi))
TWO_PI = float(2.0 * np.pi)
INV_SQRT2 = float(1.0 / np.sqrt(2.0))

_ACT_SET = "natural_log_exp_and_others"  # contains exp, ln AND square


class _pinned_act_table_set:
    """Context manager pinning all activations to one table set.

    The stock insert_act_table_loads maps each function to the FIRST set
    containing it (exp/square -> exp_and_others, ln -> natural_log_exp...),
    which forces a ~1.3us ACT_TABLE_LOAD at nearly every function switch
    (21 loads, ~27us of Scalar-engine time for this kernel).  Exp, Ln and
    Square all live in natural_log_exp_and_others, so present every other
    set as empty while compiling; indices (act_func_set_id) are preserved
    so walrus still resolves the right table.  The original function is
    restored on exit so no framework state stays mutated.
    """

    def __enter__(self):
        import concourse.hw_specs as hw_specs

        real_fn = hw_specs.get_activation_tables
        self._saved = bacc.get_activation_tables

        def pinned(module_arch):
            real = real_fn(module_arch)
            assert _ACT_SET in real, sorted(real)
            return {
                name: (fns if name == _ACT_SET else set())
                for name, fns in real.items()
            }

        bacc.get_activation_tables = pinned

    def __exit__(self, *exc):
        bacc.get_activation_tables = self._saved
        return False


def _tile_sizes(r: int, f: int) -> list[int]:
    """Tile sizes summing to r: small tiles at BOTH ends (fast pipeline
    fill and drain), full-size `f` tiles in the middle."""
    taper = [f // 4, f // 4, f // 2]
    if r < 2 * sum(taper) + f or r % f != 0:
        g = min(f, r)
        while r % g:
            g //= 2
        return [g] * (r // g)
    mid = r - 2 * sum(taper)
    return taper + [f] * (mid // f) + taper[::-1]


def build_nc(bc: int = BC, f: int = 2048) -> bass.Bass:
    """Build the per-core Bass program for a shard of `bc` samples.

    `f` = samples per partition per full tile. Requires bc % (P*f) == 0.
    """
    r = bc // P           # samples per partition
    assert bc % P == 0 and r % f == 0, (bc, f)
    sizes = _tile_sizes(r, f)
    assert sum(sizes) == r

    f32 = mybir.dt.float32
    bf16 = mybir.dt.bfloat16
    Exp = mybir.ActivationFunctionType.Exp
    Ln = mybir.ActivationFunctionType.Ln
    Square = mybir.ActivationFunctionType.Square
    Alu = mybir.AluOpType

    nc = bacc.Bacc()

    preds = nc.dram_tensor("preds", [bc, 4], f32, kind="ExternalInput")
    truth = nc.dram_tensor("truth", [bc, 1], f32, kind="ExternalInput")
    loss = nc.dram_tensor("loss", [bc], f32, kind="ExternalOutput")

    # Partition p owns the contiguous sample range [p*r, (p+1)*r) so every
    # DMA moves one contiguous chunk per partition.
    preds_v = preds[:].rearrange("(p r) c -> p r c", p=P)   # [128, r, 4]
    truth_v = truth[:].rearrange("(p r) c -> p (r c)", p=P)  # [128, r]
    loss_v = loss[:].rearrange("(p r) -> p r", p=P)          # [128, r]

    with tile.TileContext(nc) as tc:
        with (
            tc.tile_pool(name="io", bufs=2) as io_pool,
            tc.tile_pool(name="tmp", bufs=2) as tmp,
        ):
            pos = 0
            for ftile in sizes:
                sl = slice(pos, pos + ftile)
                pos += ftile
                pt = io_pool.tile([P, f, 4], f32, tag="preds", name="pt")[
                    :, :ftile, :
                ]
                nc.sync.dma_start(out=pt[:], in_=preds_v[:, sl, :])
                yt = io_pool.tile([P, f], f32, tag="truth", name="yt")[:, :ftile]
                nc.sync.dma_start(out=yt[:], in_=truth_v[:, sl])

                m1 = pt[:, :, 0]
                l1 = pt[:, :, 1]
                m2 = pt[:, :, 2]
                l2 = pt[:, :, 3]

                # Interval-colored scratch tiles; each hosts several
                # short-lived values per iteration (lifetimes disjoint,
                # never in-place).
                ca = tmp.tile([P, f], f32, tag="ca", name="ca")[:, :ftile]
                cb = tmp.tile([P, f], f32, tag="cb", name="cb")[:, :ftile]
                cc = tmp.tile([P, f], f32, tag="cc", name="cc")[:, :ftile]
                cd = tmp.tile([P, f], f32, tag="cd", name="cd")[:, :ftile]
                ce = tmp.tile([P, f], f32, tag="ce", name="ce")[:, :ftile]
                ot = io_pool.tile([P, f], f32, tag="ot", name="ot")[:, :ftile]

                nc.scalar.activation(ca[:], l1, Exp, scale=2.0)    # ca = sigma1^2
                nc.scalar.activation(cb[:], l2, Exp, scale=2.0)    # cb = sigma2^2
                nc.vector.tensor_add(cc[:], ca[:], cb[:])          # cc = s
                nc.scalar.activation(cd[:], yt[:], Ln)             # cd = logy
                nc.vector.tensor_add(ca[:], m1, m2)                # ca = mu
                nc.vector.tensor_sub(cb[:], cd[:], ca[:])          # cb = z = logy-mu
                nc.vector.reciprocal_approx_fast(ca[:], cc[:])     # ca = 1/s (~51 ULP)
                nc.scalar.activation(ce[:], cb[:], Square, scale=INV_SQRT2)
                #   ce = z^2/2
                nc.scalar.activation(cb[:], cc[:], Ln, scale=TWO_PI)  # cb = ln(2pi*s)
                nc.vector.scalar_tensor_tensor(                    # yt = 0.5*ln(2pi*s)+logy
                    yt[:], cb[:], 0.5, cd[:], Alu.mult, Alu.add
                )
                nc.vector.tensor_mul(cc[:], ce[:], ca[:])          # cc = z^2/(2s)
                nc.vector.tensor_add(ot[:], yt[:], cc[:])          # ot = loss
                nc.sync.dma_start(out=loss_v[:, sl], in_=ot[:])

    with _pinned_act_table_set():
        nc.compile()
    return nc


def build_nc_raw(bc: int = BC, f: int = 1024) -> bass.Bass:
    """Raw-bass (no Tile) build: hand-placed semaphores, explicit software
    pipeline with 3-deep load prefetch.  Per tile i:

      SP :  load pt_i, yt_i       (waits only sb >= i-3: loads run 3 ahead)
      ACT:  A_i = e1, e2, lny     (after tile i's DMAs)
            C_{i-1} = q, h2, r2   (after B_{i-1})
      DVE:  B_i = s, mu, z        (after A_i)
            D_{i-1} = w, v, ot    (after C_{i-1})
      GPS:  store ot_{i-1}        (after D_{i-1}; SWDGE so SP never blocks)

    pt/yt have 4 slots, everything else 3.  C/D-stage values reuse the
    dead A/B-stage slots of their own tile (h2->e1, q->e2, r2->mu, w->z,
    v->s): each WAR is covered by an existing semaphore wait or by DVE
    program order.  v = Square(z*sqrt(pi)) * exp(-ln(2pi s)) = z^2/(2s),
    so every constant is an immediate scale.
    """
    r = bc // P
    assert bc % P == 0 and r % f == 0, (bc, f)
    if r // f >= 4:
        # One half-tile at each end: the first A-stage starts after half a
        # DMA, and the final C/D/store chain is half-length.
        sizes = [f // 2] + [f] * (r // f - 1) + [f // 2]
    else:
        sizes = [f] * (r // f)
    nt = len(sizes)
    assert nt >= 5, nt
    offs = [sum(sizes[:i]) for i in range(nt)]

    f32 = mybir.dt.float32
    Exp = mybir.ActivationFunctionType.Exp
    Ln = mybir.ActivationFunctionType.Ln
    Square = mybir.ActivationFunctionType.Square
    Alu = mybir.AluOpType
    SQRT_PI = float(np.sqrt(np.pi))

    # The sim race detector does not credit same-engine program order for
    # raw (non-Tile) blocks; engines execute their streams in order on HW
    # (Tile itself relies on this), so it is disabled here.  Cross-engine
    # ordering is enforced by the explicit semaphores below.
    nc = bacc.Bacc(detect_race_conditions=False)
    preds = nc.dram_tensor("preds", [bc, 4], f32, kind="ExternalInput")
    truth = nc.dram_tensor("truth", [bc, 1], f32, kind="ExternalInput")
    loss = nc.dram_tensor("loss", [bc], f32, kind="ExternalOutput")

    preds_v = preds[:].rearrange("(p r) c -> p r c", p=P)
    truth_v = truth[:].rearrange("(p r) c -> p (r c)", p=P)
    loss_v = loss[:].rearrange("(p r) -> p r", p=P)

    from contextlib import ExitStack

    NPT = 4   # pt/yt slots (3-deep load prefetch)
    NSC = 4   # scratch/ot/cd slots

    with ExitStack() as ctx:
        pt = [
            ctx.enter_context(nc.sbuf_tensor(f"pt{k}", [P, f, 4], f32))
            for k in range(NPT)
        ]
        yt = ctx.enter_context(nc.sbuf_tensor("yt", [P, NPT, f], f32))
        ot = ctx.enter_context(nc.sbuf_tensor("ot", [P, NSC, f], f32))
        e1 = ctx.enter_context(nc.sbuf_tensor("e1", [P, NSC, f], f32))
        e2 = ctx.enter_context(nc.sbuf_tensor("e2", [P, NSC, f], f32))
        st = ctx.enter_context(nc.sbuf_tensor("st", [P, NSC, f], f32))
        mu = ctx.enter_context(nc.sbuf_tensor("mu", [P, NSC, f], f32))
        zt = ctx.enter_context(nc.sbuf_tensor("zt", [P, NSC, f], f32))
        cd = ctx.enter_context(nc.sbuf_tensor("cd", [P, NSC, f], f32))
        # DMA sems are slot-split: each transfer lands as 16 separate +1s,
        # so concurrent transfers must never share a semaphore threshold.
        # Loads run up to 3 tiles ahead -> one dsem per pt slot.
        dsem = [
            ctx.enter_context(nc.semaphore(f"dsem{k}")) for k in range(NPT)
        ]
        osem = [
            ctx.enter_context(nc.semaphore(f"osem{k}")) for k in range(NSC)
        ]
        sa = ctx.enter_context(nc.semaphore("sa"))
        sb = ctx.enter_context(nc.semaphore("sb"))
        sc = ctx.enter_context(nc.semaphore("sc"))
        sd = ctx.enter_context(nc.semaphore("sd"))
        # The gpsimd stream already ends by waiting for every store to
        # complete (osem totals), so the expensive Q7 drain at block exit
        # is redundant.
        block = ctx.enter_context(nc.Block(no_gpsimd_drain=True))

        def views(i):
            p4 = i % NPT
            p3 = i % NSC
            n = sizes[i]
            return {
                "n": n,
                "sl": slice(offs[i], offs[i] + n),
                "pt": pt[p4],
                "m1": pt[p4][:, :n, 0],
                "l1": pt[p4][:, :n, 1],
                "m2": pt[p4][:, :n, 2],
                "l2": pt[p4][:, :n, 3],
                "yt": yt[:, p4, :n],
                "ot": ot[:, p3, :n],
                "e1": e1[:, p3, :n],
                "e2": e2[:, p3, :n],
                "s": st[:, p3, :n],
                "mu": mu[:, p3, :n],
                "z": zt[:, p3, :n],
                "lny": cd[:, p3, :n],
                "h2": e1[:, p3, :n],   # ln(2pi*s)  (C_i; e1 dead after B_i)
                "q": e2[:, p3, :n],    # pi * z^2   (C_i; e2 dead after B_i)
                "r2": mu[:, p3, :n],   # 1/(2pi*s)  (C_i; mu dead after B_i)
                "w": zt[:, p3, :n],    # 0.5*h2+lny (D_i; z dead after C_i)
                "v": st[:, p3, :n],    # z^2/(2s)   (D_i; s dead after C_i)
            }

        @block.sync
        def _(sync):
            # Loads ONLY (stores are on GpSimd/SWDGE): SP never blocks on
            # compute progress except to recycle slots 4 tiles back.
            for i in range(nt):
                if i >= NPT:
                    sync.wait_ge(sb, i - NPT + 1)  # B_{i-4} done: slot free
                v = views(i)
                sync.dma_start(
                    out=v["pt"][:, : v["n"], :], in_=preds_v[:, v["sl"], :]
                ).then_inc(dsem[i % NPT], 16)
                sync.dma_start(out=v["yt"], in_=truth_v[:, v["sl"]]).then_inc(
                    dsem[i % NPT], 16
                )

        @block.gpsimd
        def _(gpsimd):
            for j in range(nt):
                gpsimd.wait_ge(sd, j + 1)
                v2 = views(j)
                gpsimd.dma_start(out=loss_v[:, v2["sl"]], in_=v2["ot"]).then_inc(
                    osem[j % NSC], 16
                )
            for p in range(NSC):
                total = len([j for j in range(nt) if j % NSC == p])
                gpsimd.wait_ge(osem[p], 16 * total)

        @block.scalar
        def _(scalar):
            for i in range(nt):
                # --- A_i ---
                scalar.wait_ge(dsem[i % NPT], 32 * (i // NPT + 1))
                if i >= NSC:
                    # sd >= k implies sc >= k implies sb >= k (D waits C
                    # waits B), so one wait covers e1/e2/lny/h2 slot reuse.
                    scalar.wait_ge(sd, i - NSC + 1)
                v = views(i)
                nc.scalar.activation(v["e1"], v["l1"], Exp, scale=2.0)
                nc.scalar.activation(v["e2"], v["l2"], Exp, scale=2.0)
                nc.scalar.activation(v["lny"], v["yt"], Ln).then_inc(sa)
                # --- C_{i-1} ---
                if i >= 1:
                    scalar.wait_ge(sb, i)  # z_{i-1}, s_{i-1} ready
                    u = views(i - 1)
                    nc.scalar.activation(u["q"], u["z"], Square, scale=SQRT_PI)
                    nc.scalar.activation(u["h2"], u["s"], Ln, scale=TWO_PI)
                    nc.scalar.activation(
                        u["r2"], u["h2"], Exp, scale=-1.0
                    ).then_inc(sc)
            scalar.wait_ge(sb, nt)
            u = views(nt - 1)
            nc.scalar.activation(u["q"], u["z"], Square, scale=SQRT_PI)
            nc.scalar.activation(u["h2"], u["s"], Ln, scale=TWO_PI)
            nc.scalar.activation(u["r2"], u["h2"], Exp, scale=-1.0).then_inc(sc)

        @block.vector
        def _(vector):
            for i in range(nt):
                # --- B_i ---
                vector.wait_ge(sa, i + 1)
                if i >= NSC:
                    vector.wait_ge(sc, i - NSC + 1)  # s/z slot (C_{i-3} read)
                v = views(i)
                nc.vector.tensor_add(v["s"], v["e1"], v["e2"])
                nc.vector.tensor_add(v["mu"], v["m1"], v["m2"])
                nc.vector.tensor_sub(v["z"], v["lny"], v["mu"]).then_inc(sb)
                # --- D_{i-1} ---
                if i >= 1:
                    vector.wait_ge(sc, i)  # q/h2/r2 ready
                    j = i - 1
                    if j >= NSC:
                        # ot slot free once store_{j-3} (same slot) done.
                        vector.wait_ge(osem[j % NSC], 16 * (j // NSC))
                    u = views(j)
                    nc.vector.scalar_tensor_tensor(
                        u["w"], u["h2"], 0.5, u["lny"], Alu.mult, Alu.add
                    )
                    nc.vector.tensor_mul(u["v"], u["q"], u["r2"])  # z^2/(2s)
                    nc.vector.tensor_add(u["ot"], u["w"], u["v"]).then_inc(sd)
            vector.wait_ge(sc, nt)
            j = nt - 1
            if j >= NSC:
                vector.wait_ge(osem[j % NSC], 16 * (j // NSC))
            u = views(j)
            nc.vector.scalar_tensor_tensor(
                u["w"], u["h2"], 0.5, u["lny"], Alu.mult, Alu.add
            )
            nc.vector.tensor_mul(u["v"], u["q"], u["r2"])
            nc.vector.tensor_add(u["ot"], u["w"], u["v"]).then_inc(sd)

    with _pinned_act_table_set():
        nc.compile()
    return nc


_NC = None


def _get_nc() -> bass.Bass:
    global _NC
    if _NC is None:
        _NC = build_nc_raw()
    return _NC


def kernel(preds: np.ndarray, truth: np.ndarray) -> np.ndarray:
    assert preds.shape == (B, 4) and truth.shape == (B, 1)
    nc = _get_nc()
    preds = np.ascontiguousarray(preds, dtype=np.float32)
    truth = np.ascontiguousarray(truth, dtype=np.float32)
    in_maps = [
        {
            "preds": preds[c * BC : (c + 1) * BC],
            "truth": truth[c * BC : (c + 1) * BC],
        }
        for c in range(N_CORES)
    ]
    res = run_bass_kernel_spmd(nc, in_maps, core_ids=list(range(N_CORES)))
    return np.concatenate([res.results[c]["loss"] for c in range(N_CORES)], axis=0)


# revision 57
# speedup vs baseline: 1.0054x; 1.0054x over previous
"""Log-normal NLL loss kernel for Trainium2 (8 NeuronCores, data-parallel).

Reference math (per sample):
    preds = (mu1, log_sigma1, mu2, log_sigma2); y = truth
    s   = exp(2*log_sigma1) + exp(2*log_sigma2)          # sigma1^2 + sigma2^2
    mu  = mu1 + mu2
    out = log(y) + 0.5*log(2*pi*s) + (log(y) - mu)^2 / (2*s)

Batch is split evenly across the 8 cores (no communication).  The per-core
program (build_nc_raw) is raw bass with hand-placed semaphores: a 4-stage
software pipeline (DMA-load / ACT / DVE / store-on-GpSimd) with 3-deep
load prefetch.  The division by s runs on ACT via exp/-ln
(v = Square(z*sqrt(pi)) * exp(-ln(2*pi*s)) = z^2/(2*s)): 6 ACT passes +
6 DVE passes per element, balanced against the HBM roofline
(~24 MB/core @ ~358 GB/s ~= 67 us; measured ~92-108 us end to end).

build_nc is an earlier TileContext implementation kept as a reference /
fallback (~108-115 us).
"""

import os
import sys

import numpy as np

for _p in ("/opt/trn_rl_repo", os.path.expanduser("~/.axon_site/_ro/trn_rl_repo")):
    if os.path.isdir(_p) and _p not in sys.path:
        sys.path.insert(0, _p)

import concourse.bacc as bacc
import concourse.bass as bass
import concourse.mybir as mybir
import concourse.tile as tile
from concourse.bass_utils import run_bass_kernel_spmd

B = 8388608
N_CORES = 8
BC = B // N_CORES  # 1048576 samples per core
P = 128            # SBUF partitions

LOG_2PI = float(np.log(2.0 * np.pi))
TWO_PI = float(2.0 * np.pi)
INV_SQRT2 = float(1.0 / np.sqrt(2.0))

_ACT_SET = "natural_log_exp_and_others"  # contains exp, ln AND square


class _pinned_act_table_set:
    """Context manager pinning all activations to one table set.

    The stock insert_act_table_loads maps each function to the FIRST set
    containing it (exp/square -> exp_and_others, ln -> natural_log_exp...),
    which forces a ~1.3us ACT_TABLE_LOAD at nearly every function switch
    (21 loads, ~27us of Scalar-engine time for this kernel).  Exp, Ln and
    Square all live in natural_log_exp_and_others, so present every other
    set as empty while compiling; indices (act_func_set_id) are preserved
    so walrus still resolves the right table.  The original function is
    restored on exit so no framework state stays mutated.
    """

    def __enter__(self):
        import concourse.hw_specs as hw_specs

        real_fn = hw_specs.get_activation_tables
        self._saved = bacc.get_activation_tables

        def pinned(module_arch):
            real = real_fn(module_arch)
            assert _ACT_SET in real, sorted(real)
            return {
                name: (fns if name == _ACT_SET else set())
                for name, fns in real.items()
            }

        bacc.get_activation_tables = pinned

    def __exit__(self, *exc):
        bacc.get_activation_tables = self._saved
        return False


def _tile_sizes(r: int, f: int) -> list[int]:
    """Tile sizes summing to r: small tiles at BOTH ends (fast pipeline
    fill and drain), full-size `f` tiles in the middle."""
    taper = [f // 4, f // 4, f // 2]
    if r < 2 * sum(taper) + f or r % f != 0:
        g = min(f, r)
        while r % g:
            g //= 2
        return [g] * (r // g)
    mid = r - 2 * sum(taper)
    return taper + [f] * (mid // f) + taper[::-1]


def build_nc(bc: int = BC, f: int = 2048) -> bass.Bass:
    """Build the per-core Bass program for a shard of `bc` samples.

    `f` = samples per partition per full tile. Requires bc % (P*f) == 0.
    """
    r = bc // P           # samples per partition
    assert bc % P == 0 and r % f == 0, (bc, f)
    sizes = _tile_sizes(r, f)
    assert sum(sizes) == r

    f32 = mybir.dt.float32
    bf16 = mybir.dt.bfloat16
    Exp = mybir.ActivationFunctionType.Exp
    Ln = mybir.ActivationFunctionType.Ln
    Square = mybir.ActivationFunctionType.Square
    Alu = mybir.AluOpType

    nc = bacc.Bacc()

    preds = nc.dram_tensor("preds", [bc, 4], f32, kind="ExternalInput")
    truth = nc.dram_tensor("truth", [bc, 1], f32, kind="ExternalInput")
    loss = nc.dram_tensor("loss", [bc], f32, kind="ExternalOutput")

    # Partition p owns the contiguous sample range [p*r, (p+1)*r) so every
    # DMA moves one contiguous chunk per partition.
    preds_v = preds[:].rearrange("(p r) c -> p r c", p=P)   # [128, r, 4]
    truth_v = truth[:].rearrange("(p r) c -> p (r c)", p=P)  # [128, r]
    loss_v = loss[:].rearrange("(p r) -> p r", p=P)          # [128, r]

    with tile.TileContext(nc) as tc:
        with (
            tc.tile_pool(name="io", bufs=2) as io_pool,
            tc.tile_pool(name="tmp", bufs=2) as tmp,
        ):
            pos = 0
            for ftile in sizes:
                sl = slice(pos, pos + ftile)
                pos += ftile
                pt = io_pool.tile([P, f, 4], f32, tag="preds", name="pt")[
                    :, :ftile, :
                ]
                nc.sync.dma_start(out=pt[:], in_=preds_v[:, sl, :])
                yt = io_pool.tile([P, f], f32, tag="truth", name="yt")[:, :ftile]
                nc.sync.dma_start(out=yt[:], in_=truth_v[:, sl])

                m1 = pt[:, :, 0]
                l1 = pt[:, :, 1]
                m2 = pt[:, :, 2]
                l2 = pt[:, :, 3]

                # Interval-colored scratch tiles; each hosts several
                # short-lived values per iteration (lifetimes disjoint,
                # never in-place).
                ca = tmp.tile([P, f], f32, tag="ca", name="ca")[:, :ftile]
                cb = tmp.tile([P, f], f32, tag="cb", name="cb")[:, :ftile]
                cc = tmp.tile([P, f], f32, tag="cc", name="cc")[:, :ftile]
                cd = tmp.tile([P, f], f32, tag="cd", name="cd")[:, :ftile]
                ce = tmp.tile([P, f], f32, tag="ce", name="ce")[:, :ftile]
                ot = io_pool.tile([P, f], f32, tag="ot", name="ot")[:, :ftile]

                nc.scalar.activation(ca[:], l1, Exp, scale=2.0)    # ca = sigma1^2
                nc.scalar.activation(cb[:], l2, Exp, scale=2.0)    # cb = sigma2^2
                nc.vector.tensor_add(cc[:], ca[:], cb[:])          # cc = s
                nc.scalar.activation(cd[:], yt[:], Ln)             # cd = logy
                nc.vector.tensor_add(ca[:], m1, m2)                # ca = mu
                nc.vector.tensor_sub(cb[:], cd[:], ca[:])          # cb = z = logy-mu
                nc.vector.reciprocal_approx_fast(ca[:], cc[:])     # ca = 1/s (~51 ULP)
                nc.scalar.activation(ce[:], cb[:], Square, scale=INV_SQRT2)
                #   ce = z^2/2
                nc.scalar.activation(cb[:], cc[:], Ln, scale=TWO_PI)  # cb = ln(2pi*s)
                nc.vector.scalar_tensor_tensor(                    # yt = 0.5*ln(2pi*s)+logy
                    yt[:], cb[:], 0.5, cd[:], Alu.mult, Alu.add
                )
                nc.vector.tensor_mul(cc[:], ce[:], ca[:])          # cc = z^2/(2s)
                nc.vector.tensor_add(ot[:], yt[:], cc[:])          # ot = loss
                nc.sync.dma_start(out=loss_v[:, sl], in_=ot[:])

    with _pinned_act_table_set():
        nc.compile()
    return nc


def build_nc_raw(bc: int = BC, f: int = 1024) -> bass.Bass:
    """Raw-bass (no Tile) build: hand-placed semaphores, explicit software
    pipeline with 3-deep load prefetch.  Per tile i:

      SP :  load pt_i, yt_i       (waits only sb >= i-3: loads run 3 ahead)
      ACT:  A_i = e1, e2, lny     (after tile i's DMAs)
            C_{i-1} = q, h2, r2   (after B_{i-1})
      DVE:  B_i = s, mu, z        (after A_i)
            D_{i-1} = w, v, ot    (after C_{i-1})
      GPS:  store ot_{i-1}        (after D_{i-1}; SWDGE so SP never blocks)

    pt/yt have 4 slots, everything else 3.  C/D-stage values reuse the
    dead A/B-stage slots of their own tile (h2->e1, q->e2, r2->mu, w->z,
    v->s): each WAR is covered by an existing semaphore wait or by DVE
    program order.  v = Square(z*sqrt(pi)) * exp(-ln(2pi s)) = z^2/(2s),
    so every constant is an immediate scale.
    """
    r = bc // P
    assert bc % P == 0 and r % f == 0, (bc, f)
    if r // f >= 4:
        # One half-tile at each end: the first A-stage starts after half a
        # DMA, and the final C/D/store chain is half-length.
        sizes = [f // 2] + [f] * (r // f - 1) + [f // 2]
    else:
        sizes = [f] * (r // f)
    nt = len(sizes)
    assert nt >= 5, nt
    offs = [sum(sizes[:i]) for i in range(nt)]

    f32 = mybir.dt.float32
    Exp = mybir.ActivationFunctionType.Exp
    Ln = mybir.ActivationFunctionType.Ln
    Square = mybir.ActivationFunctionType.Square
    Alu = mybir.AluOpType
    SQRT_PI = float(np.sqrt(np.pi))

    # The sim race detector does not credit same-engine program order for
    # raw (non-Tile) blocks; engines execute their streams in order on HW
    # (Tile itself relies on this), so it is disabled here.  Cross-engine
    # ordering is enforced by the explicit semaphores below.
    nc = bacc.Bacc(detect_race_conditions=False)
    preds = nc.dram_tensor("preds", [bc, 4], f32, kind="ExternalInput")
    truth = nc.dram_tensor("truth", [bc, 1], f32, kind="ExternalInput")
    loss = nc.dram_tensor("loss", [bc], f32, kind="ExternalOutput")

    preds_v = preds[:].rearrange("(p r) c -> p r c", p=P)
    truth_v = truth[:].rearrange("(p r) c -> p (r c)", p=P)
    loss_v = loss[:].rearrange("(p r) -> p r", p=P)

    from contextlib import ExitStack

    NPT = 4   # pt/yt slots (3-deep load prefetch)
    NSC = 4   # scratch/ot/cd slots

    with ExitStack() as ctx:
        pt = [
            ctx.enter_context(nc.sbuf_tensor(f"pt{k}", [P, f, 4], f32))
            for k in range(NPT)
        ]
        yt = ctx.enter_context(nc.sbuf_tensor("yt", [P, NPT, f], f32))
        ot = ctx.enter_context(nc.sbuf_tensor("ot", [P, NSC, f], f32))
        e1 = ctx.enter_context(nc.sbuf_tensor("e1", [P, NSC, f], f32))
        e2 = ctx.enter_context(nc.sbuf_tensor("e2", [P, NSC, f], f32))
        st = ctx.enter_context(nc.sbuf_tensor("st", [P, NSC, f], f32))
        mu = ctx.enter_context(nc.sbuf_tensor("mu", [P, NSC, f], f32))
        zt = ctx.enter_context(nc.sbuf_tensor("zt", [P, NSC, f], f32))
        cd = ctx.enter_context(nc.sbuf_tensor("cd", [P, NSC, f], f32))
        # DMA sems are slot-split: each transfer lands as 16 separate +1s,
        # so concurrent transfers must never share a semaphore threshold.
        # Loads run up to 3 tiles ahead -> one dsem per pt slot.
        dsem = [
            ctx.enter_context(nc.semaphore(f"dsem{k}")) for k in range(NPT)
        ]
        osem = [
            ctx.enter_context(nc.semaphore(f"osem{k}")) for k in range(NSC)
        ]
        sa = ctx.enter_context(nc.semaphore("sa"))
        sb = ctx.enter_context(nc.semaphore("sb"))
        sc = ctx.enter_context(nc.semaphore("sc"))
        sd = ctx.enter_context(nc.semaphore("sd"))
        osp = ctx.enter_context(nc.semaphore("osp"))  # SP-issued tail stores
        # The gpsimd stream already ends by waiting for every store to
        # complete (osem totals), so the expensive Q7 drain at block exit
        # is redundant.
        block = ctx.enter_context(nc.Block(no_gpsimd_drain=True))

        def views(i):
            p4 = i % NPT
            p3 = i % NSC
            n = sizes[i]
            return {
                "n": n,
                "sl": slice(offs[i], offs[i] + n),
                "pt": pt[p4],
                "m1": pt[p4][:, :n, 0],
                "l1": pt[p4][:, :n, 1],
                "m2": pt[p4][:, :n, 2],
                "l2": pt[p4][:, :n, 3],
                "yt": yt[:, p4, :n],
                "ot": ot[:, p3, :n],
                "e1": e1[:, p3, :n],
                "e2": e2[:, p3, :n],
                "s": st[:, p3, :n],
                "mu": mu[:, p3, :n],
                "z": zt[:, p3, :n],
                "lny": cd[:, p3, :n],
                "h2": e1[:, p3, :n],   # ln(2pi*s)  (C_i; e1 dead after B_i)
                "q": e2[:, p3, :n],    # pi * z^2   (C_i; e2 dead after B_i)
                "r2": mu[:, p3, :n],   # 1/(2pi*s)  (C_i; mu dead after B_i)
                "w": zt[:, p3, :n],    # 0.5*h2+lny (D_i; z dead after C_i)
                "v": st[:, p3, :n],    # z^2/(2s)   (D_i; s dead after C_i)
            }

        @block.sync
        def _(sync):
            # Loads ONLY (stores are on GpSimd/SWDGE): SP never blocks on
            # compute progress except to recycle slots 4 tiles back.
            for i in range(nt):
                if i >= NPT:
                    sync.wait_ge(sb, i - NPT + 1)  # B_{i-4} done: slot free
                v = views(i)
                sync.dma_start(
                    out=v["pt"][:, : v["n"], :], in_=preds_v[:, v["sl"], :]
                ).then_inc(dsem[i % NPT], 16)
                sync.dma_start(out=v["yt"], in_=truth_v[:, v["sl"]]).then_inc(
                    dsem[i % NPT], 16
                )
            # Last two stores on SP's faster HWDGE path (SP is idle by
            # then); dedicated sem -- SWDGE and HWDGE cannot share one.
            for j in (nt - 2, nt - 1):
                sync.wait_ge(sd, j + 1)
                v2 = views(j)
                sync.dma_start(out=loss_v[:, v2["sl"]], in_=v2["ot"]).then_inc(
                    osp, 16
                )
            sync.wait_ge(osp, 32)

        @block.gpsimd
        def _(gpsimd):
            for j in range(nt - 2):
                gpsimd.wait_ge(sd, j + 1)
                v2 = views(j)
                gpsimd.dma_start(out=loss_v[:, v2["sl"]], in_=v2["ot"]).then_inc(
                    osem[j % NSC], 16
                )
            for p in range(NSC):
                total = len([j for j in range(nt - 2) if j % NSC == p])
                gpsimd.wait_ge(osem[p], 16 * total)

        @block.scalar
        def _(scalar):
            for i in range(nt):
                # --- A_i ---
                scalar.wait_ge(dsem[i % NPT], 32 * (i // NPT + 1))
                if i >= NSC:
                    # sd >= k implies sc >= k implies sb >= k (D waits C
                    # waits B), so one wait covers e1/e2/lny/h2 slot reuse.
                    scalar.wait_ge(sd, i - NSC + 1)
                v = views(i)
                nc.scalar.activation(v["e1"], v["l1"], Exp, scale=2.0)
                nc.scalar.activation(v["e2"], v["l2"], Exp, scale=2.0)
                nc.scalar.activation(v["lny"], v["yt"], Ln).then_inc(sa)
                # --- C_{i-1} ---
                if i >= 1:
                    scalar.wait_ge(sb, i)  # z_{i-1}, s_{i-1} ready
                    u = views(i - 1)
                    nc.scalar.activation(u["q"], u["z"], Square, scale=SQRT_PI)
                    nc.scalar.activation(u["h2"], u["s"], Ln, scale=TWO_PI)
                    nc.scalar.activation(
                        u["r2"], u["h2"], Exp, scale=-1.0
                    ).then_inc(sc)
            scalar.wait_ge(sb, nt)
            u = views(nt - 1)
            nc.scalar.activation(u["q"], u["z"], Square, scale=SQRT_PI)
            nc.scalar.activation(u["h2"], u["s"], Ln, scale=TWO_PI)
            nc.scalar.activation(u["r2"], u["h2"], Exp, scale=-1.0).then_inc(sc)

        @block.vector
        def _(vector):
            for i in range(nt):
                # --- B_i ---
                vector.wait_ge(sa, i + 1)
                if i >= NSC:
                    vector.wait_ge(sc, i - NSC + 1)  # s/z slot (C_{i-3} read)
                v = views(i)
                nc.vector.tensor_add(v["s"], v["e1"], v["e2"])
                nc.vector.tensor_add(v["mu"], v["m1"], v["m2"])
                nc.vector.tensor_sub(v["z"], v["lny"], v["mu"]).then_inc(sb)
                # --- D_{i-1} ---
                if i >= 1:
                    vector.wait_ge(sc, i)  # q/h2/r2 ready
                    j = i - 1
                    if j >= NSC:
                        # ot slot free once store_{j-3} (same slot) done.
                        vector.wait_ge(osem[j % NSC], 16 * (j // NSC))
                    u = views(j)
                    nc.vector.scalar_tensor_tensor(
                        u["w"], u["h2"], 0.5, u["lny"], Alu.mult, Alu.add
                    )
                    nc.vector.tensor_mul(u["v"], u["q"], u["r2"])  # z^2/(2s)
                    nc.vector.tensor_add(u["ot"], u["w"], u["v"]).then_inc(sd)
            vector.wait_ge(sc, nt)
            j = nt - 1
            if j >= NSC:
                vector.wait_ge(osem[j % NSC], 16 * (j // NSC))
            u = views(j)
            nc.vector.scalar_tensor_tensor(
                u["w"], u["h2"], 0.5, u["lny"], Alu.mult, Alu.add
            )
            nc.vector.tensor_mul(u["v"], u["q"], u["r2"])
            nc.vector.tensor_add(u["ot"], u["w"], u["v"]).then_inc(sd)

    with _pinned_act_table_set():
        nc.compile()
    return nc


_NC = None


def _get_nc() -> bass.Bass:
    global _NC
    if _NC is None:
        _NC = build_nc_raw()
    return _NC


def kernel(preds: np.ndarray, truth: np.ndarray) -> np.ndarray:
    assert preds.shape == (B, 4) and truth.shape == (B, 1)
    nc = _get_nc()
    preds = np.ascontiguousarray(preds, dtype=np.float32)
    truth = np.ascontiguousarray(truth, dtype=np.float32)
    in_maps = [
        {
            "preds": preds[c * BC : (c + 1) * BC],
            "truth": truth[c * BC : (c + 1) * BC],
        }
        for c in range(N_CORES)
    ]
    res = run_bass_kernel_spmd(nc, in_maps, core_ids=list(range(N_CORES)))
    return np.concatenate([res.results[c]["loss"] for c in range(N_CORES)], axis=0)


# revision 58
# speedup vs baseline: 1.0582x; 1.0525x over previous
"""Log-normal NLL loss kernel for Trainium2 (8 NeuronCores, data-parallel).

Reference math (per sample):
    preds = (mu1, log_sigma1, mu2, log_sigma2); y = truth
    s   = exp(2*log_sigma1) + exp(2*log_sigma2)          # sigma1^2 + sigma2^2
    mu  = mu1 + mu2
    out = log(y) + 0.5*log(2*pi*s) + (log(y) - mu)^2 / (2*s)

Batch is split evenly across the 8 cores (no communication).  The per-core
program (build_nc_raw) is raw bass with hand-placed semaphores: a 4-stage
software pipeline (DMA-load / ACT / DVE / store-on-GpSimd) with 3-deep
load prefetch.  The division by s runs on ACT via exp/-ln
(v = Square(z*sqrt(pi)) * exp(-ln(2*pi*s)) = z^2/(2*s)): 6 ACT passes +
6 DVE passes per element, balanced against the HBM roofline
(~24 MB/core @ ~358 GB/s ~= 67 us; measured ~92-108 us end to end).

build_nc is an earlier TileContext implementation kept as a reference /
fallback (~108-115 us).
"""

import os
import sys

import numpy as np

for _p in ("/opt/trn_rl_repo", os.path.expanduser("~/.axon_site/_ro/trn_rl_repo")):
    if os.path.isdir(_p) and _p not in sys.path:
        sys.path.insert(0, _p)

import concourse.bacc as bacc
import concourse.bass as bass
import concourse.mybir as mybir
import concourse.tile as tile
from concourse.bass_utils import run_bass_kernel_spmd

B = 8388608
N_CORES = 8
BC = B // N_CORES  # 1048576 samples per core
P = 128            # SBUF partitions

LOG_2PI = float(np.log(2.0 * np.pi))
TWO_PI = float(2.0 * np.pi)
INV_SQRT2 = float(1.0 / np.sqrt(2.0))

_ACT_SET = "natural_log_exp_and_others"  # contains exp, ln AND square


class _pinned_act_table_set:
    """Context manager pinning all activations to one table set.

    The stock insert_act_table_loads maps each function to the FIRST set
    containing it (exp/square -> exp_and_others, ln -> natural_log_exp...),
    which forces a ~1.3us ACT_TABLE_LOAD at nearly every function switch
    (21 loads, ~27us of Scalar-engine time for this kernel).  Exp, Ln and
    Square all live in natural_log_exp_and_others, so present every other
    set as empty while compiling; indices (act_func_set_id) are preserved
    so walrus still resolves the right table.  The original function is
    restored on exit so no framework state stays mutated.
    """

    def __enter__(self):
        import concourse.hw_specs as hw_specs

        real_fn = hw_specs.get_activation_tables
        self._saved = bacc.get_activation_tables

        def pinned(module_arch):
            real = real_fn(module_arch)
            assert _ACT_SET in real, sorted(real)
            return {
                name: (fns if name == _ACT_SET else set())
                for name, fns in real.items()
            }

        bacc.get_activation_tables = pinned

    def __exit__(self, *exc):
        bacc.get_activation_tables = self._saved
        return False


def _tile_sizes(r: int, f: int) -> list[int]:
    """Tile sizes summing to r: small tiles at BOTH ends (fast pipeline
    fill and drain), full-size `f` tiles in the middle."""
    taper = [f // 4, f // 4, f // 2]
    if r < 2 * sum(taper) + f or r % f != 0:
        g = min(f, r)
        while r % g:
            g //= 2
        return [g] * (r // g)
    mid = r - 2 * sum(taper)
    return taper + [f] * (mid // f) + taper[::-1]


def build_nc(bc: int = BC, f: int = 2048) -> bass.Bass:
    """Build the per-core Bass program for a shard of `bc` samples.

    `f` = samples per partition per full tile. Requires bc % (P*f) == 0.
    """
    r = bc // P           # samples per partition
    assert bc % P == 0 and r % f == 0, (bc, f)
    sizes = _tile_sizes(r, f)
    assert sum(sizes) == r

    f32 = mybir.dt.float32
    bf16 = mybir.dt.bfloat16
    Exp = mybir.ActivationFunctionType.Exp
    Ln = mybir.ActivationFunctionType.Ln
    Square = mybir.ActivationFunctionType.Square
    Alu = mybir.AluOpType

    nc = bacc.Bacc()

    preds = nc.dram_tensor("preds", [bc, 4], f32, kind="ExternalInput")
    truth = nc.dram_tensor("truth", [bc, 1], f32, kind="ExternalInput")
    loss = nc.dram_tensor("loss", [bc], f32, kind="ExternalOutput")

    # Partition p owns the contiguous sample range [p*r, (p+1)*r) so every
    # DMA moves one contiguous chunk per partition.
    preds_v = preds[:].rearrange("(p r) c -> p r c", p=P)   # [128, r, 4]
    truth_v = truth[:].rearrange("(p r) c -> p (r c)", p=P)  # [128, r]
    loss_v = loss[:].rearrange("(p r) -> p r", p=P)          # [128, r]

    with tile.TileContext(nc) as tc:
        with (
            tc.tile_pool(name="io", bufs=2) as io_pool,
            tc.tile_pool(name="tmp", bufs=2) as tmp,
        ):
            pos = 0
            for ftile in sizes:
                sl = slice(pos, pos + ftile)
                pos += ftile
                pt = io_pool.tile([P, f, 4], f32, tag="preds", name="pt")[
                    :, :ftile, :
                ]
                nc.sync.dma_start(out=pt[:], in_=preds_v[:, sl, :])
                yt = io_pool.tile([P, f], f32, tag="truth", name="yt")[:, :ftile]
                nc.sync.dma_start(out=yt[:], in_=truth_v[:, sl])

                m1 = pt[:, :, 0]
                l1 = pt[:, :, 1]
                m2 = pt[:, :, 2]
                l2 = pt[:, :, 3]

                # Interval-colored scratch tiles; each hosts several
                # short-lived values per iteration (lifetimes disjoint,
                # never in-place).
                ca = tmp.tile([P, f], f32, tag="ca", name="ca")[:, :ftile]
                cb = tmp.tile([P, f], f32, tag="cb", name="cb")[:, :ftile]
                cc = tmp.tile([P, f], f32, tag="cc", name="cc")[:, :ftile]
                cd = tmp.tile([P, f], f32, tag="cd", name="cd")[:, :ftile]
                ce = tmp.tile([P, f], f32, tag="ce", name="ce")[:, :ftile]
                ot = io_pool.tile([P, f], f32, tag="ot", name="ot")[:, :ftile]

                nc.scalar.activation(ca[:], l1, Exp, scale=2.0)    # ca = sigma1^2
                nc.scalar.activation(cb[:], l2, Exp, scale=2.0)    # cb = sigma2^2
                nc.vector.tensor_add(cc[:], ca[:], cb[:])          # cc = s
                nc.scalar.activation(cd[:], yt[:], Ln)             # cd = logy
                nc.vector.tensor_add(ca[:], m1, m2)                # ca = mu
                nc.vector.tensor_sub(cb[:], cd[:], ca[:])          # cb = z = logy-mu
                nc.vector.reciprocal_approx_fast(ca[:], cc[:])     # ca = 1/s (~51 ULP)
                nc.scalar.activation(ce[:], cb[:], Square, scale=INV_SQRT2)
                #   ce = z^2/2
                nc.scalar.activation(cb[:], cc[:], Ln, scale=TWO_PI)  # cb = ln(2pi*s)
                nc.vector.scalar_tensor_tensor(                    # yt = 0.5*ln(2pi*s)+logy
                    yt[:], cb[:], 0.5, cd[:], Alu.mult, Alu.add
                )
                nc.vector.tensor_mul(cc[:], ce[:], ca[:])          # cc = z^2/(2s)
                nc.vector.tensor_add(ot[:], yt[:], cc[:])          # ot = loss
                nc.sync.dma_start(out=loss_v[:, sl], in_=ot[:])

    with _pinned_act_table_set():
        nc.compile()
    return nc


def build_nc_raw(bc: int = BC, f: int = 1024) -> bass.Bass:
    """Raw-bass (no Tile) build: hand-placed semaphores, explicit software
    pipeline with 3-deep load prefetch.  Per tile i:

      SP :  load pt_i, yt_i       (waits only sb >= i-3: loads run 3 ahead)
      ACT:  A_i = e1, e2, lny     (after tile i's DMAs)
            C_{i-1} = q, h2, r2   (after B_{i-1})
      DVE:  B_i = s, mu, z        (after A_i)
            D_{i-1} = w, v, ot    (after C_{i-1})
      GPS:  store ot_{i-1}        (after D_{i-1}; SWDGE so SP never blocks)

    pt/yt have 4 slots, everything else 3.  C/D-stage values reuse the
    dead A/B-stage slots of their own tile (h2->e1, q->e2, r2->mu, w->z,
    v->s): each WAR is covered by an existing semaphore wait or by DVE
    program order.  v = Square(z*sqrt(pi)) * exp(-ln(2pi s)) = z^2/(2s),
    so every constant is an immediate scale.
    """
    r = bc // P
    assert bc % P == 0 and r % f == 0, (bc, f)
    if r // f >= 4:
        # One half-tile at each end: the first A-stage starts after half a
        # DMA, and the final C/D/store chain is half-length.
        sizes = [f // 2] + [f] * (r // f - 1) + [f // 2]
    else:
        sizes = [f] * (r // f)
    nt = len(sizes)
    assert nt >= 5, nt
    offs = [sum(sizes[:i]) for i in range(nt)]

    f32 = mybir.dt.float32
    Exp = mybir.ActivationFunctionType.Exp
    Ln = mybir.ActivationFunctionType.Ln
    Square = mybir.ActivationFunctionType.Square
    Alu = mybir.AluOpType
    SQRT_PI = float(np.sqrt(np.pi))

    # The sim race detector does not credit same-engine program order for
    # raw (non-Tile) blocks; engines execute their streams in order on HW
    # (Tile itself relies on this), so it is disabled here.  Cross-engine
    # ordering is enforced by the explicit semaphores below.
    nc = bacc.Bacc(detect_race_conditions=False)
    preds = nc.dram_tensor("preds", [bc, 4], f32, kind="ExternalInput")
    truth = nc.dram_tensor("truth", [bc, 1], f32, kind="ExternalInput")
    loss = nc.dram_tensor("loss", [bc], f32, kind="ExternalOutput")

    preds_v = preds[:].rearrange("(p r) c -> p r c", p=P)
    truth_v = truth[:].rearrange("(p r) c -> p (r c)", p=P)
    loss_v = loss[:].rearrange("(p r) -> p r", p=P)

    from contextlib import ExitStack

    NPT = 4   # pt/yt slots (3-deep load prefetch)
    NSC = 4   # scratch/ot/cd slots

    with ExitStack() as ctx:
        pt = [
            ctx.enter_context(nc.sbuf_tensor(f"pt{k}", [P, f, 4], f32))
            for k in range(NPT)
        ]
        yt = ctx.enter_context(nc.sbuf_tensor("yt", [P, NPT, f], f32))
        ot = ctx.enter_context(nc.sbuf_tensor("ot", [P, NSC, f], f32))
        e1 = ctx.enter_context(nc.sbuf_tensor("e1", [P, NSC, f], f32))
        e2 = ctx.enter_context(nc.sbuf_tensor("e2", [P, NSC, f], f32))
        st = ctx.enter_context(nc.sbuf_tensor("st", [P, NSC, f], f32))
        mu = ctx.enter_context(nc.sbuf_tensor("mu", [P, NSC, f], f32))
        zt = ctx.enter_context(nc.sbuf_tensor("zt", [P, NSC, f], f32))
        cd = ctx.enter_context(nc.sbuf_tensor("cd", [P, NSC, f], f32))
        # DMA sems are slot-split: each transfer lands as 16 separate +1s,
        # so concurrent transfers must never share a semaphore threshold.
        # Loads run up to 3 tiles ahead -> one dsem per pt slot.
        dsem = [
            ctx.enter_context(nc.semaphore(f"dsem{k}")) for k in range(NPT)
        ]
        osem = [
            ctx.enter_context(nc.semaphore(f"osem{k}")) for k in range(NSC)
        ]
        sa = ctx.enter_context(nc.semaphore("sa"))
        sb = ctx.enter_context(nc.semaphore("sb"))
        sc = ctx.enter_context(nc.semaphore("sc"))
        sd = ctx.enter_context(nc.semaphore("sd"))
        # The gpsimd stream already ends by waiting for every store to
        # complete (osem totals), so the expensive Q7 drain at block exit
        # is redundant.
        block = ctx.enter_context(nc.Block(no_gpsimd_drain=True))

        def views(i):
            p4 = i % NPT
            p3 = i % NSC
            n = sizes[i]
            return {
                "n": n,
                "sl": slice(offs[i], offs[i] + n),
                "pt": pt[p4],
                "m1": pt[p4][:, :n, 0],
                "l1": pt[p4][:, :n, 1],
                "m2": pt[p4][:, :n, 2],
                "l2": pt[p4][:, :n, 3],
                "yt": yt[:, p4, :n],
                "ot": ot[:, p3, :n],
                "e1": e1[:, p3, :n],
                "e2": e2[:, p3, :n],
                "s": st[:, p3, :n],
                "mu": mu[:, p3, :n],
                "z": zt[:, p3, :n],
                "lny": cd[:, p3, :n],
                "h2": e1[:, p3, :n],   # ln(2pi*s)  (C_i; e1 dead after B_i)
                "q": e2[:, p3, :n],    # pi * z^2   (C_i; e2 dead after B_i)
                "r2": mu[:, p3, :n],   # 1/(2pi*s)  (C_i; mu dead after B_i)
                "w": zt[:, p3, :n],    # 0.5*h2+lny (D_i; z dead after C_i)
                "v": st[:, p3, :n],    # z^2/(2s)   (D_i; s dead after C_i)
            }

        @block.sync
        def _(sync):
            # Loads ONLY (stores are on GpSimd/SWDGE): SP never blocks on
            # compute progress except to recycle slots 4 tiles back.
            for i in range(nt):
                if i >= NPT:
                    sync.wait_ge(sb, i - NPT + 1)  # B_{i-4} done: slot free
                v = views(i)
                sync.dma_start(
                    out=v["pt"][:, : v["n"], :], in_=preds_v[:, v["sl"], :]
                ).then_inc(dsem[i % NPT], 16)
                sync.dma_start(out=v["yt"], in_=truth_v[:, v["sl"]]).then_inc(
                    dsem[i % NPT], 16
                )

        @block.gpsimd
        def _(gpsimd):
            for j in range(nt):
                gpsimd.wait_ge(sd, j + 1)
                v2 = views(j)
                gpsimd.dma_start(out=loss_v[:, v2["sl"]], in_=v2["ot"]).then_inc(
                    osem[j % NSC], 16
                )
            for p in range(NSC):
                total = len([j for j in range(nt) if j % NSC == p])
                gpsimd.wait_ge(osem[p], 16 * total)

        @block.scalar
        def _(scalar):
            for i in range(nt):
                # --- A_i ---
                scalar.wait_ge(dsem[i % NPT], 32 * (i // NPT + 1))
                if i >= NSC:
                    # sd >= k implies sc >= k implies sb >= k (D waits C
                    # waits B), so one wait covers e1/e2/lny/h2 slot reuse.
                    scalar.wait_ge(sd, i - NSC + 1)
                v = views(i)
                nc.scalar.activation(v["e1"], v["l1"], Exp, scale=2.0)
                nc.scalar.activation(v["e2"], v["l2"], Exp, scale=2.0)
                nc.scalar.activation(v["lny"], v["yt"], Ln).then_inc(sa)
                # --- C_{i-1} ---
                if i >= 1:
                    scalar.wait_ge(sb, i)  # z_{i-1}, s_{i-1} ready
                    u = views(i - 1)
                    nc.scalar.activation(u["q"], u["z"], Square, scale=SQRT_PI)
                    nc.scalar.activation(u["h2"], u["s"], Ln, scale=TWO_PI)
                    nc.scalar.activation(
                        u["r2"], u["h2"], Exp, scale=-1.0
                    ).then_inc(sc)
            scalar.wait_ge(sb, nt)
            u = views(nt - 1)
            nc.scalar.activation(u["q"], u["z"], Square, scale=SQRT_PI)
            nc.scalar.activation(u["h2"], u["s"], Ln, scale=TWO_PI)
            nc.scalar.activation(u["r2"], u["h2"], Exp, scale=-1.0).then_inc(sc)

        @block.vector
        def _(vector):
            for i in range(nt):
                # --- B_i ---
                vector.wait_ge(sa, i + 1)
                if i >= NSC:
                    vector.wait_ge(sc, i - NSC + 1)  # s/z slot (C_{i-3} read)
                v = views(i)
                nc.vector.tensor_add(v["s"], v["e1"], v["e2"])
                nc.vector.tensor_add(v["mu"], v["m1"], v["m2"])
                nc.vector.tensor_sub(v["z"], v["lny"], v["mu"]).then_inc(sb)
                # --- D_{i-1} ---
                if i >= 1:
                    vector.wait_ge(sc, i)  # q/h2/r2 ready
                    j = i - 1
                    if j >= NSC:
                        # ot slot free once store_{j-3} (same slot) done.
                        vector.wait_ge(osem[j % NSC], 16 * (j // NSC))
                    u = views(j)
                    nc.vector.scalar_tensor_tensor(
                        u["w"], u["h2"], 0.5, u["lny"], Alu.mult, Alu.add
                    )
                    nc.vector.tensor_mul(u["v"], u["q"], u["r2"])  # z^2/(2s)
                    nc.vector.tensor_add(u["ot"], u["w"], u["v"]).then_inc(sd)
            vector.wait_ge(sc, nt)
            j = nt - 1
            if j >= NSC:
                vector.wait_ge(osem[j % NSC], 16 * (j // NSC))
            u = views(j)
            nc.vector.scalar_tensor_tensor(
                u["w"], u["h2"], 0.5, u["lny"], Alu.mult, Alu.add
            )
            nc.vector.tensor_mul(u["v"], u["q"], u["r2"])
            nc.vector.tensor_add(u["ot"], u["w"], u["v"]).then_inc(sd)

    with _pinned_act_table_set():
        nc.compile()
    return nc


_NC = None


def _get_nc() -> bass.Bass:
    global _NC
    if _NC is None:
        _NC = build_nc_raw()
    return _NC


def kernel(preds: np.ndarray, truth: np.ndarray) -> np.ndarray:
    assert preds.shape == (B, 4) and truth.shape == (B, 1)
    nc = _get_nc()
    preds = np.ascontiguousarray(preds, dtype=np.float32)
    truth = np.ascontiguousarray(truth, dtype=np.float32)
    in_maps = [
        {
            "preds": preds[c * BC : (c + 1) * BC],
            "truth": truth[c * BC : (c + 1) * BC],
        }
        for c in range(N_CORES)
    ]
    res = run_bass_kernel_spmd(nc, in_maps, core_ids=list(range(N_CORES)))
    return np.concatenate([res.results[c]["loss"] for c in range(N_CORES)], axis=0)
